# revision 1
# baseline (speedup 1.0000x reference)
"""Blockwise 8x8 2D orthonormal DCT (Dct2d) for Trainium2, 8 NeuronCores.

Input  x: (64, 1, 1024, 1024) f32  ->  Output: (64, 64, 128, 128) f32
Data parallel over the batch dim: 8 samples per core.

Per-core algorithm (per 128-row strip of each 1024x1024 image):
  mm1 (per 128-col tile t): PSUM[w, (gh,i)] = X_t^T @ C,  C = I_16 (x) A^T
      (data tile is the *stationary* operand, so the transpose is fused)
  mm2 (per tile t):         PSUM[(gh,i), (j,gw16)] = Y1_t^T @ R,
      R[(g,l),(j,g)] = A[j,l]  (permuted block-diagonal)
  strided PSUM->SBUF copies assemble [(gh,i), j, gw] so the HBM store has
  contiguous 512B runs per (channel, row).
"""

from contextlib import ExitStack

import numpy as np

import concourse.bass as bass
import concourse.tile as tile
from concourse import bacc, mybir
from concourse.bass_utils import run_bass_kernel_spmd

N_CORES = 8
H = W = 1024
N_STRIPS = H // 128  # 8


def _dct_consts(A: np.ndarray) -> tuple[np.ndarray, np.ndarray]:
    A = np.asarray(A, np.float32)
    C = np.zeros((128, 128), np.float32)
    R = np.zeros((128, 128), np.float32)
    for g in range(16):
        C[g * 8 : (g + 1) * 8, g * 8 : (g + 1) * 8] = A.T
    for g in range(16):
        for l in range(8):
            for j in range(8):
                R[g * 8 + l, j * 16 + g] = A[j, l]
    return C, R


def _build(samples: int, Cmat: np.ndarray, Rmat: np.ndarray) -> bass.Bass:
    nc = bacc.Bacc(
        "TRN2", target_bir_lowering=False, debug=False, num_devices=N_CORES
    )
    f32 = mybir.dt.float32
    x_ap = nc.dram_tensor("x", (samples, H, W), f32, kind="ExternalInput").ap()
    out_ap = nc.dram_tensor(
        "out", (samples, 64, H // 8, W // 8), f32, kind="ExternalOutput"
    ).ap()
    cd = nc.inline_tensor(Cmat, name="cmat").ap()
    rd = nc.inline_tensor(Rmat, name="rmat").ap()

    with tile.TileContext(nc) as tc, ExitStack() as ctx:
        consts = ctx.enter_context(tc.tile_pool(name="consts", bufs=1))
        xpool = ctx.enter_context(tc.tile_pool(name="xs", bufs=4))
        y1pool = ctx.enter_context(tc.tile_pool(name="y1", bufs=3))
        opool = ctx.enter_context(tc.tile_pool(name="os", bufs=3))
        ps1 = ctx.enter_context(tc.tile_pool(name="ps1", bufs=3, space="PSUM"))
        ps2 = ctx.enter_context(tc.tile_pool(name="ps2", bufs=3, space="PSUM"))

        ct = consts.tile([128, 128], f32)
        nc.sync.dma_start(ct[:], cd[:])
        rt = consts.tile([128, 128], f32)
        nc.sync.dma_start(rt[:], rd[:])

        for s in range(samples):
            for st in range(N_STRIPS):
                xt = xpool.tile([128, 1024], f32)
                nc.sync.dma_start(xt[:], x_ap[s, st * 128 : (st + 1) * 128, :])

                # columns t*128 + (gh*8+i): row-DCT'd, transposed tiles
                y1 = y1pool.tile([128, 1024], f32)
                for b in range(2):
                    p1 = ps1.tile([128, 512], f32)
                    for t4 in range(4):
                        t = b * 4 + t4
                        nc.tensor.matmul(
                            p1[:, t4 * 128 : (t4 + 1) * 128],
                            lhsT=xt[:, t * 128 : (t + 1) * 128],
                            rhs=ct[:],
                            start=(t4 == 0),
                            stop=(t4 == 3),
                        )
                    if b == 0:
                        nc.vector.tensor_copy(y1[:, 0:512], p1[:])
                    else:
                        nc.scalar.copy(y1[:, 512:1024], p1[:])

                # [p=(gh,i), j, gw]
                ot = opool.tile([128, 8, 128], f32)
                for b in range(2):
                    p2 = ps2.tile([128, 512], f32)
                    for t4 in range(4):
                        t = b * 4 + t4
                        nc.tensor.matmul(
                            p2[:, t4 * 128 : (t4 + 1) * 128],
                            lhsT=y1[:, t * 128 : (t + 1) * 128],
                            rhs=rt[:],
                            start=(t4 == 0),
                            stop=(t4 == 3),
                        )
                    # psum col (t4, j, g) -> ot[:, j, b*64 + t4*16 + g]
                    src = p2.rearrange("p (t j g) -> p t j g", t=4, j=8)
                    dst = ot[:, :, b * 64 : (b + 1) * 64].rearrange(
                        "p j (t g) -> p t j g", t=4
                    )
                    if b == 0:
                        nc.scalar.copy(dst, src)
                    else:
                        nc.vector.tensor_copy(dst, src)

                dram_view = out_ap[s, :, st * 16 : (st + 1) * 16, :].rearrange(
                    "(i j) gh gw -> gh i j gw", i=8
                )
                nc.sync.dma_start(dram_view, ot[:])

    nc.compile()
    return nc


_cache: dict = {}


def _get_program(samples: int, A: np.ndarray) -> bass.Bass:
    key = (samples, A.tobytes())
    if key not in _cache:
        C, R = _dct_consts(A)
        _cache[key] = _build(samples, C, R)
    return _cache[key]


def _run(x, A, **spmd_kwargs):
    x = np.ascontiguousarray(np.asarray(x, dtype=np.float32))
    A = np.asarray(A, dtype=np.float32)
    N = x.shape[0]
    spc = N // N_CORES  # samples per core
    nc = _get_program(spc, A)
    in_maps = [
        {"x": np.ascontiguousarray(x[i * spc : (i + 1) * spc, 0])}
        for i in range(N_CORES)
    ]
    res = run_bass_kernel_spmd(nc, in_maps, list(range(N_CORES)), **spmd_kwargs)
    out = np.concatenate(
        [res.results[i]["out"] for i in range(N_CORES)], axis=0
    )
    return out.astype(np.float32, copy=False), res


def kernel(x, A):
    out, _ = _run(x, A)
    return out



# revision 2
# speedup vs baseline: 1.4558x; 1.4558x over previous
"""Blockwise 8x8 2D orthonormal DCT (Dct2d) for Trainium2, 8 NeuronCores.

Input  x: (64, 1, 1024, 1024) f32  ->  Output: (64, 64, 128, 128) f32
Data parallel over the batch dim: 8 samples per core.

Per-core algorithm (per 128-row strip of each 1024x1024 image):
  cast:  xb = bf16(x strip)  (scalar engine; matmuls run 4x faster in bf16
         and the accuracy budget allows it)
  mm1 (per 128-col tile t): PSUM[w, (gh,i)] = Xb_t^T @ C,  C = I_16 (x) A^T
      (data tile is the *stationary* operand, so the transpose is fused)
  mm2 (per tile t):         PSUM[(gh,i), (j,gw16)] = Y1_t^T @ R,
      R[(g,l),(j,g)] = A[j,l]  (permuted block-diagonal)
  strided PSUM->SBUF copies assemble [(gh,i), j, gw] so the HBM store has
  contiguous 512B runs per (channel, row).

With bf16 matmuls the per-strip engine budget (DMA 2.9us, PE 1.7us,
DVE 2.0us, ACT 1.6us) leaves the serialized DMA device as the sole
bottleneck: ~186us/core = 64MiB / 360B/ns.
"""

from contextlib import ExitStack

import ml_dtypes
import numpy as np

import concourse.bass as bass
import concourse.tile as tile
from concourse import bacc, mybir
from concourse.bass_utils import run_bass_kernel_spmd

N_CORES = 8
H = W = 1024
N_STRIPS = H // 128  # 8


def _dct_consts(A: np.ndarray) -> tuple[np.ndarray, np.ndarray]:
    A = np.asarray(A, np.float32)
    C = np.zeros((128, 128), np.float32)
    R = np.zeros((128, 128), np.float32)
    for g in range(16):
        C[g * 8 : (g + 1) * 8, g * 8 : (g + 1) * 8] = A.T
    for g in range(16):
        for l in range(8):
            for j in range(8):
                R[g * 8 + l, j * 16 + g] = A[j, l]
    return C.astype(ml_dtypes.bfloat16), R.astype(ml_dtypes.bfloat16)


def _build(samples: int, Cmat: np.ndarray, Rmat: np.ndarray) -> bass.Bass:
    nc = bacc.Bacc(
        "TRN2", target_bir_lowering=False, debug=False, num_devices=N_CORES
    )
    f32 = mybir.dt.float32
    bf16 = mybir.dt.bfloat16
    x_ap = nc.dram_tensor("x", (samples, H, W), f32, kind="ExternalInput").ap()
    out_ap = nc.dram_tensor(
        "out", (samples, 64, H // 8, W // 8), f32, kind="ExternalOutput"
    ).ap()
    cd = nc.inline_tensor(Cmat, name="cmat").ap()
    rd = nc.inline_tensor(Rmat, name="rmat").ap()

    with tile.TileContext(nc) as tc, ExitStack() as ctx:
        consts = ctx.enter_context(tc.tile_pool(name="consts", bufs=1))
        xpool = ctx.enter_context(tc.tile_pool(name="xs", bufs=4))
        xbpool = ctx.enter_context(tc.tile_pool(name="xb", bufs=3))
        y1pool = ctx.enter_context(tc.tile_pool(name="y1", bufs=3))
        opool = ctx.enter_context(tc.tile_pool(name="os", bufs=3))
        ps1 = ctx.enter_context(tc.tile_pool(name="ps1", bufs=3, space="PSUM"))
        ps2 = ctx.enter_context(tc.tile_pool(name="ps2", bufs=3, space="PSUM"))

        ct = consts.tile([128, 128], bf16)
        nc.sync.dma_start(ct[:], cd[:])
        rt = consts.tile([128, 128], bf16)
        nc.sync.dma_start(rt[:], rd[:])

        for s in range(samples):
            for st in range(N_STRIPS):
                xt = xpool.tile([128, 1024], f32)
                nc.sync.dma_start(xt[:], x_ap[s, st * 128 : (st + 1) * 128, :])

                xb = xbpool.tile([128, 1024], bf16)
                nc.scalar.copy(xb[:], xt[:])

                # columns t*128 + (gh*8+i): row-DCT'd, transposed tiles
                y1 = y1pool.tile([128, 1024], bf16)
                for b in range(2):
                    p1 = ps1.tile([128, 512], f32)
                    for t4 in range(4):
                        t = b * 4 + t4
                        nc.tensor.matmul(
                            p1[:, t4 * 128 : (t4 + 1) * 128],
                            lhsT=xb[:, t * 128 : (t + 1) * 128],
                            rhs=ct[:],
                            start=(t4 == 0),
                            stop=(t4 == 3),
                        )
                    nc.vector.tensor_copy(
                        y1[:, b * 512 : (b + 1) * 512], p1[:]
                    )

                # [p=(gh,i), j, gw]
                ot = opool.tile([128, 8, 128], f32)
                for b in range(2):
                    p2 = ps2.tile([128, 512], f32)
                    for t4 in range(4):
                        t = b * 4 + t4
                        nc.tensor.matmul(
                            p2[:, t4 * 128 : (t4 + 1) * 128],
                            lhsT=y1[:, t * 128 : (t + 1) * 128],
                            rhs=rt[:],
                            start=(t4 == 0),
                            stop=(t4 == 3),
                        )
                    # psum col (t4, j, g) -> ot[:, j, b*64 + t4*16 + g]
                    src = p2.rearrange("p (t j g) -> p t j g", t=4, j=8)
                    dst = ot[:, :, b * 64 : (b + 1) * 64].rearrange(
                        "p j (t g) -> p t j g", t=4
                    )
                    if b == 0:
                        nc.scalar.copy(dst, src)
                    else:
                        nc.vector.tensor_copy(dst, src)

                dram_view = out_ap[s, :, st * 16 : (st + 1) * 16, :].rearrange(
                    "(i j) gh gw -> gh i j gw", i=8
                )
                nc.sync.dma_start(dram_view, ot[:])

    nc.compile()
    return nc


_cache: dict = {}


def _get_program(samples: int, A: np.ndarray) -> bass.Bass:
    key = (samples, A.tobytes())
    if key not in _cache:
        C, R = _dct_consts(A)
        _cache[key] = _build(samples, C, R)
    return _cache[key]


def _run(x, A, **spmd_kwargs):
    x = np.ascontiguousarray(np.asarray(x, dtype=np.float32))
    A = np.asarray(A, dtype=np.float32)
    N = x.shape[0]
    spc = N // N_CORES  # samples per core
    nc = _get_program(spc, A)
    in_maps = [
        {"x": np.ascontiguousarray(x[i * spc : (i + 1) * spc, 0])}
        for i in range(N_CORES)
    ]
    res = run_bass_kernel_spmd(nc, in_maps, list(range(N_CORES)), **spmd_kwargs)
    out = np.concatenate(
        [res.results[i]["out"] for i in range(N_CORES)], axis=0
    )
    return out.astype(np.float32, copy=False), res


def kernel(x, A):
    out, _ = _run(x, A)
    return out


# revision 6
# speedup vs baseline: 2.3806x; 1.6352x over previous
"""Blockwise 8x8 2D orthonormal DCT (Dct2d) for Trainium2, 8 NeuronCores.

Input  x: (64, 1, 1024, 1024) f32  ->  Output: (64, 64, 128, 128) f32
Data parallel over the batch dim: 8 samples per core.

Per-core algorithm (per 128-row strip of each 1024x1024 image):
  in-DMA: gpsimd casting DMA loads the f32 strip directly as bf16
      (matmuls run 4x faster in bf16 and the accuracy budget allows it;
      the cast-in-DMA also halves the SBUF-side transfer size)
  mm1 (per 128-col tile t): PSUM[w, (gh,i)] = Xb_t^T @ C,  C = I_16 (x) A^T
      (data tile is the *stationary* operand, so the transpose is fused)
  mm2 (per tile t):         PSUM[(gh,i), (j,gw16)] = Y1_t^T @ R,
      R[(g,l),(j,g)] = A[j,l]  (permuted block-diagonal)
  strided PSUM->SBUF copies assemble [(gh,i), j, gw] so the HBM store has
  contiguous 512B runs per (channel, row).

The 64 strips are software-pipelined in three stages, emitted per
iteration k as A(k) / C(k-4) / B(k-2):
  A: casting input DMA (Pool/SWDGE)
  B: mm1 (PE), PSUM->SBUF y1 copy w/ bf16 cast (DVE), mm2 (PE)
  C: PSUM->SBUF output-assembly copies (ACT), output DMA (SP/HWDGE)
Each engine owns ops from a single pipeline stage, so no in-order
sequencer serializes consecutive strips through the whole per-strip
dependency chain, and the input / output DMA streams are issued from
different engines so one stream's sem waits can't head-of-line block
the other. The serialized DMA device is the bottleneck.
"""

from contextlib import ExitStack

import ml_dtypes
import numpy as np

import concourse.bass as bass
import concourse.tile as tile
from concourse import bacc, mybir
from concourse.bass_utils import run_bass_kernel_spmd

N_CORES = 8
H = W = 1024
N_STRIPS = H // 128  # 8


def _dct_consts(A: np.ndarray) -> tuple[np.ndarray, np.ndarray]:
    A = np.asarray(A, np.float32)
    C = np.zeros((128, 128), np.float32)
    R = np.zeros((128, 128), np.float32)
    for g in range(16):
        C[g * 8 : (g + 1) * 8, g * 8 : (g + 1) * 8] = A.T
    for g in range(16):
        for l in range(8):
            for j in range(8):
                R[g * 8 + l, j * 16 + g] = A[j, l]
    return C.astype(ml_dtypes.bfloat16), R.astype(ml_dtypes.bfloat16)


def _build(samples: int, Cmat: np.ndarray, Rmat: np.ndarray) -> bass.Bass:
    nc = bacc.Bacc(
        "TRN2", target_bir_lowering=False, debug=False, num_devices=N_CORES
    )
    f32 = mybir.dt.float32
    bf16 = mybir.dt.bfloat16
    x_ap = nc.dram_tensor("x", (samples, H, W), f32, kind="ExternalInput").ap()
    out_ap = nc.dram_tensor(
        "out", (samples, 64, H // 8, W // 8), f32, kind="ExternalOutput"
    ).ap()
    cd = nc.inline_tensor(Cmat, name="cmat").ap()
    rd = nc.inline_tensor(Rmat, name="rmat").ap()

    T = samples * N_STRIPS  # total strips
    SKEW_B = 2  # strips between input stage A and compute stage B
    SKEW_C = 4  # strips between input stage A and output stage C

    with tile.TileContext(nc) as tc, ExitStack() as ctx:
        consts = ctx.enter_context(tc.tile_pool(name="consts", bufs=1))
        xbpool = ctx.enter_context(tc.tile_pool(name="xb", bufs=6))
        y1pool = ctx.enter_context(tc.tile_pool(name="y1", bufs=3))
        opool = ctx.enter_context(tc.tile_pool(name="os", bufs=4))
        ps1 = ctx.enter_context(tc.tile_pool(name="ps1", bufs=2, space="PSUM"))
        ps2 = ctx.enter_context(tc.tile_pool(name="ps2", bufs=6, space="PSUM"))

        ct = consts.tile([128, 128], bf16)
        nc.sync.dma_start(ct[:], cd[:])
        rt = consts.tile([128, 128], bf16)
        nc.sync.dma_start(rt[:], rd[:])

        xb_pend: dict = {}  # k -> bf16 input tile
        p2_pend: dict = {}  # k -> [two [128,512] mm2 PSUM tiles]

        for k in range(T + SKEW_C):
            # ---- stage A: load strip k, casting f32 -> bf16 in the DMA ----
            if k < T:
                s, st = divmod(k, N_STRIPS)
                xb = xbpool.tile([128, 1024], bf16)
                nc.gpsimd.dma_start(
                    xb[:], x_ap[s, st * 128 : (st + 1) * 128, :]
                )
                xb_pend[k] = xb

            # ---- stage C: assemble and store strip k-SKEW_C ----
            i = k - SKEW_C
            if 0 <= i < T:
                s, st = divmod(i, N_STRIPS)
                ot = opool.tile([128, 8, 128], f32)
                for b, p2 in enumerate(p2_pend.pop(i)):
                    # psum col (t4, j, g) -> ot[:, j, b*64 + t4*16 + g]
                    src = p2.rearrange("p (t j g) -> p t j g", t=4, j=8)
                    dst = ot[:, :, b * 64 : (b + 1) * 64].rearrange(
                        "p j (t g) -> p t j g", t=4
                    )
                    nc.scalar.copy(dst, src)
                dram_view = out_ap[s, :, st * 16 : (st + 1) * 16, :].rearrange(
                    "(i j) gh gw -> gh i j gw", i=8
                )
                nc.sync.dma_start(dram_view, ot[:])

            # ---- stage B: two DCT matmul passes for strip k-SKEW_B ----
            j = k - SKEW_B
            if 0 <= j < T:
                xb = xb_pend.pop(j)
                # columns t*128 + (gh*8+i): row-DCT'd, transposed tiles
                y1 = y1pool.tile([128, 1024], bf16)
                for b in range(2):
                    p1 = ps1.tile([128, 512], f32)
                    for t4 in range(4):
                        t = b * 4 + t4
                        nc.tensor.matmul(
                            p1[:, t4 * 128 : (t4 + 1) * 128],
                            lhsT=xb[:, t * 128 : (t + 1) * 128],
                            rhs=ct[:],
                            start=(t4 == 0),
                            stop=(t4 == 3),
                        )
                    nc.vector.tensor_copy(
                        y1[:, b * 512 : (b + 1) * 512], p1[:]
                    )
                p2s = []
                for b in range(2):
                    p2 = ps2.tile([128, 512], f32)
                    for t4 in range(4):
                        t = b * 4 + t4
                        nc.tensor.matmul(
                            p2[:, t4 * 128 : (t4 + 1) * 128],
                            lhsT=y1[:, t * 128 : (t + 1) * 128],
                            rhs=rt[:],
                            start=(t4 == 0),
                            stop=(t4 == 3),
                        )
                    p2s.append(p2)
                p2_pend[j] = p2s

    nc.compile()
    return nc


_cache: dict = {}


def _get_program(samples: int, A: np.ndarray) -> bass.Bass:
    key = (samples, A.tobytes())
    if key not in _cache:
        C, R = _dct_consts(A)
        _cache[key] = _build(samples, C, R)
    return _cache[key]


def _run(x, A, **spmd_kwargs):
    x = np.ascontiguousarray(np.asarray(x, dtype=np.float32))
    A = np.asarray(A, dtype=np.float32)
    N = x.shape[0]
    spc = N // N_CORES  # samples per core
    nc = _get_program(spc, A)
    in_maps = [
        {"x": np.ascontiguousarray(x[i * spc : (i + 1) * spc, 0])}
        for i in range(N_CORES)
    ]
    res = run_bass_kernel_spmd(nc, in_maps, list(range(N_CORES)), **spmd_kwargs)
    out = np.concatenate(
        [res.results[i]["out"] for i in range(N_CORES)], axis=0
    )
    return out.astype(np.float32, copy=False), res


def kernel(x, A):
    out, _ = _run(x, A)
    return out


# revision 22
# speedup vs baseline: 2.5096x; 1.0542x over previous
"""Blockwise 8x8 2D orthonormal DCT (Dct2d) for Trainium2, 8 NeuronCores.

Input  x: (64, 1, 1024, 1024) f32  ->  Output: (64, 64, 128, 128) f32
Data parallel over the batch dim: 8 samples per core.

Per-core algorithm (per 128-row strip of each 1024x1024 image):
  in-DMA: gpsimd casting DMA loads the f32 strip directly as bf16
      (matmuls run 4x faster in bf16 and the accuracy budget allows it;
      the cast-in-DMA also halves the SBUF-side transfer size)
  mm1 (per 128-col tile t): PSUM[w, (gh,i)] = Xb_t^T @ C,  C = I_16 (x) A^T
      (data tile is the *stationary* operand, so the transpose is fused)
  mm2 (per tile t):         PSUM[(gh,i), (j,gw16)] = Y1_t^T @ R,
      R[(g,l),(j,g)] = A[j,l]  (permuted block-diagonal)
  strided PSUM->SBUF copies assemble [(gh,i), j, gw] so the HBM store has
  contiguous 512B runs per (channel, row).

The 64 strips are software-pipelined in three stages, emitted per
iteration k as A(k) / C(k-4) / B(k-2):
  A: casting input DMA (Pool/SWDGE)
  B: mm1 (PE), PSUM->SBUF y1 copy w/ bf16 cast (DVE), mm2 (PE)
  C: PSUM->SBUF output-assembly copies (ACT), output DMA (SP/HWDGE)
Each engine owns ops from a single pipeline stage, so no in-order
sequencer serializes consecutive strips through the whole per-strip
dependency chain, and the input / output DMA streams are issued from
different engines so one stream's sem waits can't head-of-line block
the other. The serialized DMA device is the bottleneck.
"""

from contextlib import ExitStack

import ml_dtypes
import numpy as np

import concourse.bass as bass
import concourse.tile as tile
from concourse import bacc, mybir
from concourse.bass_utils import run_bass_kernel_spmd

N_CORES = 8
H = W = 1024
N_STRIPS = H // 128  # 8


def _dct_consts(A: np.ndarray) -> tuple[np.ndarray, np.ndarray]:
    A = np.asarray(A, np.float32)
    C = np.zeros((128, 128), np.float32)
    R = np.zeros((128, 128), np.float32)
    for g in range(16):
        C[g * 8 : (g + 1) * 8, g * 8 : (g + 1) * 8] = A.T
    for g in range(16):
        for l in range(8):
            for j in range(8):
                R[g * 8 + l, j * 16 + g] = A[j, l]
    return C.astype(ml_dtypes.bfloat16), R.astype(ml_dtypes.bfloat16)


def _build(samples: int, Cmat: np.ndarray, Rmat: np.ndarray) -> bass.Bass:
    nc = bacc.Bacc(
        "TRN2", target_bir_lowering=False, debug=False, num_devices=N_CORES
    )
    f32 = mybir.dt.float32
    bf16 = mybir.dt.bfloat16
    x_ap = nc.dram_tensor("x", (samples, H, W), f32, kind="ExternalInput").ap()
    out_ap = nc.dram_tensor(
        "out", (samples, 64, H // 8, W // 8), f32, kind="ExternalOutput"
    ).ap()
    cd = nc.inline_tensor(Cmat, name="cmat").ap()
    rd = nc.inline_tensor(Rmat, name="rmat").ap()

    T = samples * N_STRIPS  # total strips
    SKEW_B = 2  # strips between input stage A and compute stage B
    SKEW_C = 3  # strips between input stage A and output stage C

    with tile.TileContext(nc) as tc, ExitStack() as ctx:
        consts = ctx.enter_context(tc.tile_pool(name="consts", bufs=1))
        xbpool = ctx.enter_context(tc.tile_pool(name="xb", bufs=6))
        y1pool = ctx.enter_context(tc.tile_pool(name="y1", bufs=4))
        opool = ctx.enter_context(tc.tile_pool(name="os", bufs=8))
        ps1 = ctx.enter_context(tc.tile_pool(name="ps1", bufs=3, space="PSUM"))
        ps2 = ctx.enter_context(tc.tile_pool(name="ps2", bufs=5, space="PSUM"))

        ct = consts.tile([128, 128], bf16)
        rt = consts.tile([128, 128], bf16)

        xb_pend: dict = {}  # k -> bf16 input tile
        p2_pend: dict = {}  # k -> [two [128,512] mm2 PSUM tiles]

        for k in range(T + SKEW_C):
            # ---- stage A: load strips (k, k+1), casting f32 -> bf16 in ----
            # the DMA. Pair loads halve the per-byte SWDGE generation work
            # on Pool so descriptor gen always stays ahead of the transfers.
            if k < T and k % 2 == 0:
                s, st = divmod(k, N_STRIPS)
                xb = xbpool.tile([128, 2, 1024], bf16)
                src = x_ap[s, st * 128 : (st + 2) * 128, :].rearrange(
                    "(two p) w -> p two w", two=2
                )
                nc.gpsimd.dma_start(xb[:], src)
                xb_pend[k] = (xb, 0)
                xb_pend[k + 1] = (xb, 1)

            if k == 0:
                # After the first input DMA so the head of the (serialized)
                # DMA device pipe isn't spent on the tiny const loads.
                nc.sync.dma_start(ct[:], cd[:])
                nc.sync.dma_start(rt[:], rd[:])

            # ---- stage C: assemble and store strip k-SKEW_C ----
            i = k - SKEW_C
            if 0 <= i < T:
                s, st = divmod(i, N_STRIPS)
                ot = opool.tile([128, 8, 128], f32)
                for b, p2 in enumerate(p2_pend.pop(i)):
                    # psum col (t4, j, g) -> ot[:, j, b*64 + t4*16 + g]
                    src = p2.rearrange("p (t j g) -> p t j g", t=4, j=8)
                    dst = ot[:, :, b * 64 : (b + 1) * 64].rearrange(
                        "p j (t g) -> p t j g", t=4
                    )
                    nc.scalar.copy(dst, src)
                dram_view = out_ap[s, :, st * 16 : (st + 1) * 16, :].rearrange(
                    "(i j) gh gw -> gh i j gw", i=8
                )
                nc.sync.dma_start(dram_view, ot[:])

            # ---- stage B: two DCT matmul passes for strip k-SKEW_B ----
            j = k - SKEW_B
            if 0 <= j < T:
                xbt, sel = xb_pend.pop(j)
                xb = xbt[:, sel]
                # columns t*128 + (gh*8+i): row-DCT'd, transposed tiles
                y1 = y1pool.tile([128, 1024], bf16)
                for b in range(2):
                    p1 = ps1.tile([128, 512], f32)
                    for t4 in range(4):
                        t = b * 4 + t4
                        nc.tensor.matmul(
                            p1[:, t4 * 128 : (t4 + 1) * 128],
                            lhsT=xb[:, t * 128 : (t + 1) * 128],
                            rhs=ct[:],
                            start=(t4 == 0),
                            stop=(t4 == 3),
                        )
                    nc.vector.tensor_copy(
                        y1[:, b * 512 : (b + 1) * 512], p1[:]
                    )
                p2s = []
                for b in range(2):
                    p2 = ps2.tile([128, 512], f32)
                    for t4 in range(4):
                        t = b * 4 + t4
                        nc.tensor.matmul(
                            p2[:, t4 * 128 : (t4 + 1) * 128],
                            lhsT=y1[:, t * 128 : (t + 1) * 128],
                            rhs=rt[:],
                            start=(t4 == 0),
                            stop=(t4 == 3),
                        )
                    p2s.append(p2)
                p2_pend[j] = p2s

    nc.compile()
    return nc


_cache: dict = {}


def _get_program(samples: int, A: np.ndarray) -> bass.Bass:
    key = (samples, A.tobytes())
    if key not in _cache:
        C, R = _dct_consts(A)
        _cache[key] = _build(samples, C, R)
    return _cache[key]


def _run(x, A, **spmd_kwargs):
    x = np.ascontiguousarray(np.asarray(x, dtype=np.float32))
    A = np.asarray(A, dtype=np.float32)
    N = x.shape[0]
    spc = N // N_CORES  # samples per core
    nc = _get_program(spc, A)
    in_maps = [
        {"x": np.ascontiguousarray(x[i * spc : (i + 1) * spc, 0])}
        for i in range(N_CORES)
    ]
    res = run_bass_kernel_spmd(nc, in_maps, list(range(N_CORES)), **spmd_kwargs)
    out = np.concatenate(
        [res.results[i]["out"] for i in range(N_CORES)], axis=0
    )
    return out.astype(np.float32, copy=False), res


def kernel(x, A):
    out, _ = _run(x, A)
    return out


# revision 28
# speedup vs baseline: 2.5119x; 1.0009x over previous
"""Blockwise 8x8 2D orthonormal DCT (Dct2d) for Trainium2, 8 NeuronCores.

Input  x: (64, 1, 1024, 1024) f32  ->  Output: (64, 64, 128, 128) f32
Data parallel over the batch dim: 8 samples per core.

Per-core algorithm (per 128-row strip of each 1024x1024 image):
  in-DMA: gpsimd casting DMA loads the f32 strip directly as bf16
      (matmuls run 4x faster in bf16 and the accuracy budget allows it;
      the cast-in-DMA also halves the SBUF-side transfer size)
  mm1 (per 128-col tile t): PSUM[w, (gh,i)] = Xb_t^T @ C,  C = I_16 (x) A^T
      (data tile is the *stationary* operand, so the transpose is fused)
  mm2 (per tile t):         PSUM[(gh,i), (j,gw16)] = Y1_t^T @ R,
      R[(g,l),(j,g)] = A[j,l]  (permuted block-diagonal)
  strided PSUM->SBUF copies assemble [(gh,i), j, gw] so the HBM store has
  contiguous 512B runs per (channel, row).

The 64 strips are software-pipelined in three stages, emitted per
iteration k as A(k) / C(k-4) / B(k-2):
  A: casting input DMA (Pool/SWDGE)
  B: mm1 (PE), PSUM->SBUF y1 copy w/ bf16 cast (DVE), mm2 (PE)
  C: PSUM->SBUF output-assembly copies (ACT), output DMA (SP/HWDGE)
Each engine owns ops from a single pipeline stage, so no in-order
sequencer serializes consecutive strips through the whole per-strip
dependency chain, and the input / output DMA streams are issued from
different engines so one stream's sem waits can't head-of-line block
the other. The serialized DMA device is the bottleneck.
"""

from contextlib import ExitStack

import ml_dtypes
import numpy as np

import concourse.bass as bass
import concourse.tile as tile
from concourse import bacc, mybir
from concourse.bass_utils import run_bass_kernel_spmd

N_CORES = 8
H = W = 1024
N_STRIPS = H // 128  # 8


def _dct_consts(A: np.ndarray) -> tuple[np.ndarray, np.ndarray]:
    A = np.asarray(A, np.float32)
    C = np.zeros((128, 128), np.float32)
    R = np.zeros((128, 128), np.float32)
    for g in range(16):
        C[g * 8 : (g + 1) * 8, g * 8 : (g + 1) * 8] = A.T
    for g in range(16):
        for l in range(8):
            for j in range(8):
                R[g * 8 + l, j * 16 + g] = A[j, l]
    # single [128, 256] constant block: C in cols 0:128, R in cols 128:256,
    # so both land in SBUF with one DMA
    return np.hstack([C, R]).astype(ml_dtypes.bfloat16)


def _build(samples: int, CRmat: np.ndarray) -> bass.Bass:
    nc = bacc.Bacc(
        "TRN2", target_bir_lowering=False, debug=False, num_devices=N_CORES
    )
    f32 = mybir.dt.float32
    bf16 = mybir.dt.bfloat16
    x_ap = nc.dram_tensor("x", (samples, H, W), f32, kind="ExternalInput").ap()
    out_ap = nc.dram_tensor(
        "out", (samples, 64, H // 8, W // 8), f32, kind="ExternalOutput"
    ).ap()
    crd = nc.inline_tensor(CRmat, name="crmat").ap()

    T = samples * N_STRIPS  # total strips
    SKEW_B = 2  # strips between input stage A and compute stage B
    SKEW_C = 3  # strips between input stage A and output stage C

    with tile.TileContext(nc) as tc, ExitStack() as ctx:
        consts = ctx.enter_context(tc.tile_pool(name="consts", bufs=1))
        xbpool = ctx.enter_context(tc.tile_pool(name="xb", bufs=6))
        y1pool = ctx.enter_context(tc.tile_pool(name="y1", bufs=4))
        opool = ctx.enter_context(tc.tile_pool(name="os", bufs=8))
        ps1 = ctx.enter_context(tc.tile_pool(name="ps1", bufs=3, space="PSUM"))
        ps2 = ctx.enter_context(tc.tile_pool(name="ps2", bufs=5, space="PSUM"))

        crt = consts.tile([128, 256], bf16)
        ct = crt[:, 0:128]
        rt = crt[:, 128:256]

        xb_pend: dict = {}  # k -> bf16 input tile
        p2_pend: dict = {}  # k -> [two [128,512] mm2 PSUM tiles]

        for k in range(T + SKEW_C):
            # ---- stage A: load strips (k, k+1), casting f32 -> bf16 in ----
            # the DMA. Pair loads halve the per-byte SWDGE generation work
            # on Pool so descriptor gen always stays ahead of the transfers.
            if k < T and k % 2 == 0:
                s, st = divmod(k, N_STRIPS)
                xb = xbpool.tile([128, 2, 1024], bf16)
                src = x_ap[s, st * 128 : (st + 2) * 128, :].rearrange(
                    "(two p) w -> p two w", two=2
                )
                nc.gpsimd.dma_start(xb[:], src)
                xb_pend[k] = (xb, 0)
                xb_pend[k + 1] = (xb, 1)

            if k == 0:
                # After the first input DMA so the head of the (serialized)
                # DMA device pipe isn't spent on the tiny const load.
                nc.sync.dma_start(crt[:], crd[:])

            # ---- stage C: assemble and store strip k-SKEW_C ----
            i = k - SKEW_C
            if 0 <= i < T:
                s, st = divmod(i, N_STRIPS)
                ot = opool.tile([128, 8, 128], f32)
                for b, p2 in enumerate(p2_pend.pop(i)):
                    # psum col (t4, j, g) -> ot[:, j, b*64 + t4*16 + g]
                    src = p2.rearrange("p (t j g) -> p t j g", t=4, j=8)
                    dst = ot[:, :, b * 64 : (b + 1) * 64].rearrange(
                        "p j (t g) -> p t j g", t=4
                    )
                    nc.scalar.copy(dst, src)
                dram_view = out_ap[s, :, st * 16 : (st + 1) * 16, :].rearrange(
                    "(i j) gh gw -> gh i j gw", i=8
                )
                nc.sync.dma_start(dram_view, ot[:])

            # ---- stage B: two DCT matmul passes for strip k-SKEW_B ----
            j = k - SKEW_B
            if 0 <= j < T:
                xbt, sel = xb_pend.pop(j)
                xb = xbt[:, sel]
                # columns t*128 + (gh*8+i): row-DCT'd, transposed tiles
                y1 = y1pool.tile([128, 1024], bf16)
                for b in range(2):
                    p1 = ps1.tile([128, 512], f32)
                    for t4 in range(4):
                        t = b * 4 + t4
                        nc.tensor.matmul(
                            p1[:, t4 * 128 : (t4 + 1) * 128],
                            lhsT=xb[:, t * 128 : (t + 1) * 128],
                            rhs=ct,
                            start=(t4 == 0),
                            stop=(t4 == 3),
                        )
                    nc.vector.tensor_copy(
                        y1[:, b * 512 : (b + 1) * 512], p1[:]
                    )
                p2s = []
                for b in range(2):
                    p2 = ps2.tile([128, 512], f32)
                    for t4 in range(4):
                        t = b * 4 + t4
                        nc.tensor.matmul(
                            p2[:, t4 * 128 : (t4 + 1) * 128],
                            lhsT=y1[:, t * 128 : (t + 1) * 128],
                            rhs=rt,
                            start=(t4 == 0),
                            stop=(t4 == 3),
                        )
                    p2s.append(p2)
                p2_pend[j] = p2s

    nc.compile()
    return nc


_cache: dict = {}


def _get_program(samples: int, A: np.ndarray) -> bass.Bass:
    key = (samples, A.tobytes())
    if key not in _cache:
        _cache[key] = _build(samples, _dct_consts(A))
    return _cache[key]


def _run(x, A, **spmd_kwargs):
    x = np.ascontiguousarray(np.asarray(x, dtype=np.float32))
    A = np.asarray(A, dtype=np.float32)
    N = x.shape[0]
    spc = N // N_CORES  # samples per core
    nc = _get_program(spc, A)
    in_maps = [
        {"x": np.ascontiguousarray(x[i * spc : (i + 1) * spc, 0])}
        for i in range(N_CORES)
    ]
    res = run_bass_kernel_spmd(nc, in_maps, list(range(N_CORES)), **spmd_kwargs)
    out = np.concatenate(
        [res.results[i]["out"] for i in range(N_CORES)], axis=0
    )
    return out.astype(np.float32, copy=False), res


def kernel(x, A):
    out, _ = _run(x, A)
    return out
